# revision 2
# baseline (speedup 1.0000x reference)
"""Trainium2 Bass kernel for nn_SamplingBlock (gnn_message_passing). v4

Strategy (v3 heritage, see kernel.py docstring for v1-v3 history)
-----------------------------------------------------------------
8 cores = (batch b in 0..3) x (vertex half h in 0..1); each core owns 4096
vertices of one batch, fully data-parallel.  Host folds all weight algebra
into 9 per-tap matrices msum (see _host_prep).

v4 redesign (from the v3 HW trace: DVE 84% busy, blend tensor_tensor
1019us; PE matmuls ~600ns/instr; Act 789us of PSUM copies + muls):
  * CORNER-LAST table: gather element = [256 ch][8 corners] f16 (4 KB).
    The trilinear blend becomes
      - ONE tensor_tensor mult per fetch: gt *= w8 broadcast on the
        *middle* dim (stride-0 there keeps the last dim [1,8] packed, so
        the op runs at the 2x DVE rate; v3's stride-0-last broadcast ran
        at 1x), then
      - a packed in-place add-tree [.,8]->[.,4]->[.,2]->[.,1], all 2x
        except the last (free-1 tail).
    Blended features end up at stride-8 column positions j=0.
  * XBAR DMA transposes (InstDmaTransposeAnt, ~14ns per 16x128 tile)
    replace ALL PE transposes and the Act PSUM->SBUF copies: SBUF->SBUF
    [128 pt, 128 ch] -> featT [128 ch, 128 pt], issued on the idle SP /
    Act HWDGE queues.
  * center-tap coordinate rows [vx;vy;vz;1] come pre-transposed from the
    host (vt dram tensor), killing the width-4 PE transposes.
  * PE keeps only the main transposed-output matmuls + shift + rep16.
"""

import os
import sys

import numpy as np

for _p in ("/opt/trn_rl_repo", "/root/.axon_site/_ro/trn_rl_repo"):
    if os.path.isdir(_p) and _p not in sys.path:
        sys.path.insert(0, _p)
        break

import concourse.bacc as bacc
import concourse.bass as bass
import concourse.mybir as mybir
import concourse.tile as tile
from concourse.bass_utils import run_bass_kernel_spmd
from concourse.masks import make_identity

# ---------------------------------------------------------------- constants
B, N, C, NN = 4, 8192, 256, 8
GRID = 32
V = GRID * GRID * GRID            # 32768 rows
# corner-last element: voxel offsets for corner j = 2*zy + xh
OFFS = (0, 1, 32, 33, 1024, 1025, 1056, 1057)
VPAD = V + OFFS[-1] + 1           # padded linear-table rows for host build
NVC = N // 2                       # vertices per core = 4096
VCHUNK = 512                       # vertices per chunk
NCHUNK = NVC // VCHUNK             # 8
GPC = VCHUNK // 128                # groups (128-pt tiles) per chunk = 4
ES = 8 * C                         # gather element: 256 ch x 8 corners f16
F32 = mybir.dt.float32
F16 = mybir.dt.float16
I16 = mybir.dt.int16
ALU = mybir.AluOpType
MM_DT = F16        # matmul operand dtype (full-rate on PE)


# ------------------------------------------------------------- device program
def _emit_index_math(nc, sb, coords, npts_free, out_r16, out_w8):
    """coords: [128, npts_free, 3] f32 AP (normalized [-1,1] space, unclipped).
    Writes out_r16 [128, npts_free] int16 row indices and
    out_w8 [128, npts_free, 8] f16 corner weights (order: zy in {00,01,10,11},
    x in {lo,hi} -> w8[..., 2*zy + xh], matching OFFS)."""
    S = npts_free
    g = sb.tile([128, S, 3], F32, tag="ixg")
    # g = clip((c+1)*15.5, 0, 31)
    nc.vector.tensor_scalar(g[:], coords, 15.5, 15.5, op0=ALU.mult, op1=ALU.add)
    nc.vector.tensor_scalar(g[:], g[:], float(GRID - 1), 0.0, op0=ALU.min,
                            op1=ALU.max)
    # floor(g) robust to HW f32->int rounding mode: q = int(g); q -= (g < q)
    qi = sb.tile([128, S, 3], mybir.dt.int32, tag="ixq")
    nc.vector.tensor_copy(qi[:], g[:])
    i0 = sb.tile([128, S, 3], F32, tag="ixi")
    nc.vector.tensor_copy(i0[:], qi[:])
    frc = sb.tile([128, S, 3], F32, tag="ixf")
    nc.vector.tensor_tensor(frc[:], g[:], i0[:], op=ALU.subtract)  # g - q
    msk = sb.tile([128, S, 3], F32, tag="ixm")
    nc.vector.tensor_scalar(msk[:], frc[:], 0.0, None, op0=ALU.is_lt)
    nc.vector.tensor_tensor(i0[:], i0[:], msk[:], op=ALU.subtract)
    nc.vector.tensor_tensor(frc[:], g[:], i0[:], op=ALU.subtract)
    # r00 = z*1024 + y*32 + x   (exact in f32)
    r = sb.tile([128, S], F32, tag="ixr")
    nc.vector.tensor_scalar(r[:], i0[:, :, 2:3].squeeze(2), 1024.0, None,
                            op0=ALU.mult)
    t = sb.tile([128, S], F32, tag="ixt")
    nc.vector.tensor_scalar(t[:], i0[:, :, 1:2].squeeze(2), 32.0, None,
                            op0=ALU.mult)
    nc.vector.tensor_tensor(r[:], r[:], t[:], op=ALU.add)
    nc.vector.tensor_tensor(r[:], r[:], i0[:, :, 0:1].squeeze(2), op=ALU.add)
    nc.vector.tensor_copy(out_r16, r[:])
    # weights: a=fx, b=fy, c=fz
    inv = sb.tile([128, S, 3], F32, tag="ixv")   # 1-f
    nc.vector.tensor_scalar(inv[:], frc[:], -1.0, 1.0, op0=ALU.mult, op1=ALU.add)
    wzy = sb.tile([128, S, 4], F32, tag="ixw")
    # zy order: 00:(1-fy)(1-fz) 01:fy(1-fz) 10:(1-fy)fz 11:fy*fz
    yz = [(inv, inv), (frc, inv), (inv, frc), (frc, frc)]
    for k, (ysrc, zsrc) in enumerate(yz):
        nc.vector.tensor_tensor(
            wzy[:, :, k : k + 1].squeeze(2),
            ysrc[:, :, 1:2].squeeze(2),
            zsrc[:, :, 2:3].squeeze(2),
            op=ALU.mult,
        )
    for k in range(4):
        nc.vector.tensor_tensor(
            out_w8[:, :, 2 * k : 2 * k + 1].squeeze(2),
            wzy[:, :, k : k + 1].squeeze(2),
            inv[:, :, 0:1].squeeze(2), op=ALU.mult)
        nc.vector.tensor_tensor(
            out_w8[:, :, 2 * k + 1 : 2 * k + 2].squeeze(2),
            wzy[:, :, k : k + 1].squeeze(2),
            frc[:, :, 0:1].squeeze(2), op=ALU.mult)


def build_program(nvc=NVC):
    nchunk = nvc // VCHUNK
    nc = bacc.Bacc("TRN2", target_bir_lowering=False, debug=False)

    verts_d = nc.dram_tensor("verts", [nvc, 3], F32, kind="ExternalInput")
    vt_d = nc.dram_tensor("vt", [4, nvc], F16, kind="ExternalInput")
    table_d = nc.dram_tensor("table", [V * ES], F16, kind="ExternalInput")
    msum_a_d = nc.dram_tensor("msum_a", [128, 9, C], MM_DT, kind="ExternalInput")
    msum_b_d = nc.dram_tensor("msum_b", [128, 9, C], MM_DT, kind="ExternalInput")
    msum_c_d = nc.dram_tensor("msum_c", [4, C], MM_DT, kind="ExternalInput")
    wsh_a_d = nc.dram_tensor("wsh_a", [128, 3 * NN], MM_DT, kind="ExternalInput")
    wsh_b_d = nc.dram_tensor("wsh_b", [128, 3 * NN], MM_DT, kind="ExternalInput")
    wsh_c_d = nc.dram_tensor("wsh_c", [4, 3 * NN], MM_DT, kind="ExternalInput")
    rep16_d = nc.dram_tensor("rep16", [16, 128], F32, kind="ExternalInput")
    # transposed output: [out-half, 128 out-ch, pts]; host re-transposes
    out_d = nc.dram_tensor("out", [2, 128, nvc], F32, kind="ExternalOutput")

    tbl = bass.AP(table_d, 0, [[ES, V], [1, ES]])

    with tile.TileContext(nc) as tc:
        with (
            tc.tile_pool(name="const", bufs=1) as cst,
            tc.tile_pool(name="wts", bufs=1) as wp,
            tc.tile_pool(name="ix", bufs=3) as ixp,
            tc.tile_pool(name="gat", bufs=6) as gp,
            tc.tile_pool(name="feat", bufs=3) as fp,
            tc.tile_pool(name="misc", bufs=2) as mp,
            tc.tile_pool(name="dram", bufs=2, space="DRAM") as dp,
            tc.tile_pool(name="pso", bufs=1, space="PSUM") as pso,
            tc.tile_pool(name="pst", bufs=2, space="PSUM") as pstp,
            tc.tile_pool(name="pss", bufs=1, space="PSUM") as pss,
            tc.tile_pool(name="psr", bufs=1, space="PSUM") as psr,
        ):
            identh = cst.tile([128, 128], F16)
            make_identity(nc, identh[:])
            msum_a = cst.tile([128, 9, C], MM_DT)
            msum_b = cst.tile([128, 9, C], MM_DT)
            msum_c = cst.tile([4, C], MM_DT)
            wsh_a = cst.tile([128, 3 * NN], MM_DT)
            wsh_b = cst.tile([128, 3 * NN], MM_DT)
            wsh_c = cst.tile([4, 3 * NN], MM_DT)
            rep16 = cst.tile([16, 128], F32)
            nc.sync.dma_start(msum_a[:], msum_a_d[:])
            nc.sync.dma_start(msum_b[:], msum_b_d[:])
            nc.sync.dma_start(msum_c[:], msum_c_d[:])
            nc.sync.dma_start(wsh_a[:], wsh_a_d[:])
            nc.sync.dma_start(wsh_b[:], wsh_b_d[:])
            nc.sync.dma_start(wsh_c[:], wsh_c_d[:])
            nc.sync.dma_start(rep16[:], rep16_d[:])

            verts = cst.tile([128, nvc // 128, 3], F32)
            nc.sync.dma_start(
                verts[:], verts_d[:].rearrange("(vt p) c -> p vt c", p=128))

            # ---- whole-core center index math ----
            r16c = wp.tile([128, nvc // 128], I16)
            w8c = wp.tile([128, nvc // 128, 8], F16)
            _emit_index_math(nc, wp, verts[:], nvc // 128, r16c[:], w8c[:])
            scr_c = dp.tile([nvc], I16)
            nc.sync.dma_start(
                scr_c[:].rearrange("(vt p) -> p vt", p=128), r16c[:])

            def load_idx(scr_ap, nidx):
                """nidx indices from DRAM scratch -> wrapped-16 idx tile,
                replicated across all 16-partition groups via rep16 matmul."""
                t16 = ixp.tile([16, nidx // 16], I16, tag=f"idx16_{nidx}")
                nc.sync.dma_start(
                    t16[:], scr_ap.rearrange("(m q) -> q m", q=16))
                f16t = ixp.tile([16, nidx // 16], F32, tag=f"idxf_{nidx}")
                nc.vector.tensor_copy(f16t[:], t16[:])
                pr = psr.tile([128, 128], F32, space="PSUM", tag="rep",
                              name="pr")
                nc.tensor.matmul(pr[:, : nidx // 16], rep16[:], f16t[:],
                                 start=True, stop=True)
                it = ixp.tile([128, nidx // 16], I16, tag=f"idx_{nidx}")
                nc.vector.tensor_copy(it[:], pr[:, : nidx // 16])
                return it

            def gather(idx_t, nidx, tag, pool):
                """one gather; elem = corner-last 8-voxel stencil (4KB);
                out col i//128 = set*GPC + g for idx i = set*512 + g*128 + p."""
                gt = pool.tile([128, nidx // 128, ES], F16, tag=tag, name="gt")
                nc.gpsimd.dma_gather(
                    gt[:], tbl, idx_t[:], nidx, nidx, ES)
                return gt

            def blend(gt, nsets, wap):
                """In-place partial trilinear blend on DVE: gt [128, ng, 2048]
                f16 viewed as [128, ng, C, 8]; wap [128, ng, 8] f16.  mult8 at
                the 2x DVE rate (broadcast on the middle dim keeps the last
                dim packed), then ONE packed run-4 add.  The remaining 4-way
                sum rides the PE transposes (PSUM accumulation)."""
                ng = nsets * GPC
                g4 = gt[:].rearrange("p g (c j) -> p g c j", j=8)
                wb = wap.unsqueeze(2).to_broadcast([128, ng, C, 8])
                nc.vector.tensor_tensor(g4, g4, wb, op=ALU.mult)
                nc.vector.tensor_tensor(
                    g4[:, :, :, 0:4], g4[:, :, :, 0:4], g4[:, :, :, 4:8],
                    op=ALU.add)
                return g4

            def transpose_into(g4, gi, pst_h, g):
                """4 accumulating transposes per ch-half via REGULAR matmul
                (lhsT = data slice, rhs = identity) -> f32 PSUM cols g*128.
                (f16-PSUM is_transpose accumulation is numerically broken on
                HW; f32 regular-matmul accumulation is exact.)"""
                for h in range(2):
                    for j in range(4):
                        nc.tensor.matmul(
                            pst_h[h][:, g * 128 : (g + 1) * 128],
                            g4[:, gi, h * 128 : (h + 1) * 128, j : j + 1
                               ].squeeze(2),
                            identh[:], start=(j == 0),
                            stop=(j == 3), skip_group_check=True)

            def centers_fetch(vc):
                idx_c = load_idx(scr_c[vc * VCHUNK : (vc + 1) * VCHUNK], VCHUNK)
                return gather(idx_c, VCHUNK, "g", gp)

            def centers_compute(vc, gtc):
                # ================= centers =================
                g4c = blend(gtc, 1, w8c[:, vc * GPC : (vc + 1) * GPC, :])
                pstc = [pstp.tile([128, VCHUNK], F32, space="PSUM",
                                  tag=f"pt{h}", name=f"ptc{h}")
                        for h in range(2)]
                for g in range(GPC):
                    transpose_into(g4c, g, pstc, g)
                ftc = fp.tile([128, 2 * VCHUNK], MM_DT, tag="fc", name="fc")
                for h in range(2):
                    nc.scalar.copy(ftc[:, h * 512 : (h + 1) * 512],
                                   pstc[h][:])
                ftc2 = fp.tile([4, VCHUNK], MM_DT, tag="fc2", name="fc2")
                nc.sync.dma_start(
                    ftc2[:], vt_d[:, vc * VCHUNK : (vc + 1) * VCHUNK])
                # ncoord nn-major: [128, NN, GPC, 3]
                ncoord = mp.tile([128, NN, GPC, 3], F32, tag="ncrd")
                for g in range(GPC):
                    vt = vc * GPC + g
                    # shift matmul -> [128 pts, 24]
                    sps = pss.tile([128, 3 * NN], F32, space="PSUM", tag="sh")
                    for ch, rhs in enumerate((wsh_a, wsh_b, wsh_c)):
                        lhs = (ftc[:, ch * 512 + g * 128 : ch * 512 + (g + 1) * 128]
                               if ch < 2 else
                               ftc2[:4, g * 128 : (g + 1) * 128])
                        nc.tensor.matmul(
                            sps[:], lhs, rhs[:], start=(ch == 0),
                            stop=(ch == 2))
                    ssb = mp.tile([128, 3 * NN], F32, tag="ssb")
                    nc.scalar.copy(ssb[:], sps[:])
                    # neighbour coords: verts + shift  [128, NN, 1, 3]
                    nc.vector.tensor_tensor(
                        ncoord[:, :, g, :],
                        ssb[:].rearrange("p (nn c) -> p nn c", c=3),
                        verts[:, vt : vt + 1, :].to_broadcast([128, NN, 3]),
                        op=ALU.add)
                # ============ neighbour index math (whole chunk) ============
                # nn-major point order: combined index = nn*GPC + g
                r16n = ixp.tile([128, NN * GPC], I16, tag="r16n")
                w8n = ixp.tile([128, NN * GPC, 8], F16, tag="w8n")
                _emit_index_math(
                    nc, ixp,
                    ncoord[:].rearrange("p nn g c -> p (nn g) c"),
                    NN * GPC, r16n[:], w8n[:])
                scr_n = dp.tile([NN * VCHUNK], I16, tag="scrn")
                nc.sync.dma_start(
                    scr_n[:].rearrange("(nn g p) -> p (nn g)", nn=NN, p=128),
                    r16n[:])
                return vc, ftc, ftc2, scr_n, w8n

            def nn_fetch(st, t):
                vc, ftc, ftc2, scr_n, w8n = st
                idx_n = load_idx(
                    scr_n[t * VCHUNK : (t + 1) * VCHUNK], VCHUNK)
                return gather(idx_n, VCHUNK, "g", gp)

            def neighbours_phase(st, gts):
                vc, ftc, ftc2, scr_n, w8n = st
                # transposed-output accumulators: [128 out-ch, 512 pts]
                out_ps = [
                    pso.tile([128, VCHUNK], F32, space="PSUM", tag=f"oT{h}",
                             name=f"ops{vc}_{h}")
                    for h in range(2)
                ]
                # main matmul tap 0 (deferred from centers_compute)
                for h in range(2):
                    hs = slice(h * 128, (h + 1) * 128)
                    nc.tensor.matmul(out_ps[h][:], msum_a[:, 0, hs],
                                     ftc[:, 0:512], start=True, stop=False)
                    nc.tensor.matmul(out_ps[h][:], msum_b[:, 0, hs],
                                     ftc[:, 512:1024], start=False, stop=False)
                    nc.tensor.matmul(out_ps[h][:], msum_c[:, hs], ftc2[:4, :],
                                     start=False, stop=False)
                # ================= neighbours (1 tap per gather) ============
                for nn_i in range(NN):
                    if nn_i + 3 < NN:
                        gts.append(nn_fetch(st, nn_i + 3))
                    gtn = gts[nn_i]
                    g4n = blend(
                        gtn, 1, w8n[:, nn_i * GPC : (nn_i + 1) * GPC, :])
                    pstn = [pstp.tile([128, VCHUNK], F32, space="PSUM",
                                      tag=f"pt{h}", name=f"ptn{h}")
                            for h in range(2)]
                    for g in range(GPC):
                        transpose_into(g4n, g, pstn, g)
                    ftn = fp.tile([128, 2 * VCHUNK], MM_DT, tag="fn",
                                  name="fn")
                    for h in range(2):
                        nc.scalar.copy(ftn[:, h * 512 : (h + 1) * 512],
                                       pstn[h][:])
                    for h in range(2):
                        hs = slice(h * 128, (h + 1) * 128)
                        nc.tensor.matmul(
                            out_ps[h][:], msum_a[:, nn_i + 1, hs],
                            ftn[:, 0:512], start=False, stop=False)
                        nc.tensor.matmul(
                            out_ps[h][:], msum_b[:, nn_i + 1, hs],
                            ftn[:, 512:1024], start=False,
                            stop=(nn_i == NN - 1))
                # ================= epilogue =================
                for h in range(2):
                    osb = mp.tile([128, VCHUNK], F32, tag="osb")
                    nc.scalar.copy(osb[:], out_ps[h][:])
                    nc.sync.dma_start(
                        out_d[h, :, vc * VCHUNK : (vc + 1) * VCHUNK], osb[:])

            # software pipeline: centers run one chunk ahead of neighbours.
            st = None
            gts = None
            gc_cur = centers_fetch(0)
            for vc in range(nchunk):
                stc = centers_compute(vc, gc_cur)
                if vc + 1 < nchunk:
                    gc_cur = centers_fetch(vc + 1)
                if st is not None:
                    neighbours_phase(st, gts)
                gts = [nn_fetch(stc, 0), nn_fetch(stc, 1), nn_fetch(stc, 2)]
                st = stc
            neighbours_phase(st, gts)

    nc.compile()
    return nc


# --------------------------------------------------------------- host wrapper
_CACHED = {}


def _host_prep(x, W_shift, b_shift, W_diff, b_diff, W_center, b_center,
               W_sum, b_sum):
    # corner-last channel-major fp16 table per batch: element for row r =
    # [ch, j] -> vol[r + OFFS[j], ch]
    xt = np.ascontiguousarray(
        np.transpose(x.reshape(B, C, V), (0, 2, 1))).astype(np.float16)
    pad = np.zeros((B, VPAD - V, C), np.float16)
    xtp = np.concatenate([xt, pad], axis=1)                # [B, VPAD, C]
    table = np.stack([xtp[:, off : off + V] for off in OFFS], axis=3)
    table = np.ascontiguousarray(table).reshape(B, V * ES)  # [B, V*2048]

    M = np.einsum("ock,cd->okd", W_sum.astype(np.float64),
                  W_diff.astype(np.float64))                  # [256, 9, 259]
    M = np.transpose(M, (1, 0, 2))                            # [9, 256, 259]
    M = M.copy()
    M[0] += W_center.astype(np.float64)
    bias = (W_sum.astype(np.float64).sum(-1) @ b_diff.astype(np.float64)
            + b_sum + b_center)                               # [256]
    # fold neighbour coordinate rows into M_0 (linear in [xp; v; 1])
    Wsh64 = W_shift.astype(np.float64)
    bsh64 = b_shift.astype(np.float64)
    for k in range(1, 9):
        Mc = M[k][:, C : C + 3]                               # [256, 3]
        M[0][:, :C] += Mc @ Wsh64[3 * (k - 1) : 3 * k, :]
        M[0][:, C : C + 3] += Mc
        bias += Mc @ bsh64[3 * (k - 1) : 3 * k]
    msum = np.zeros((9, C + 4, C), np.float16)
    for k in range(9):
        msum[k, : C + 3, :] = M[k].T.astype(np.float16)
    msum[0, C + 3, :] = bias.astype(np.float16)
    msum_a = np.ascontiguousarray(np.transpose(msum[:, 0:128, :], (1, 0, 2)))
    msum_b = np.ascontiguousarray(np.transpose(msum[:, 128:256, :], (1, 0, 2)))
    msum_c = np.ascontiguousarray(msum[0, 256:260, :])        # [4, 256], k=0

    wsh = np.zeros((C + 4, 3 * NN), np.float16)
    wsh[0:C, :] = W_shift.T.astype(np.float16)
    wsh[C + 3, :] = b_shift.astype(np.float16)
    return table, msum_a, msum_b, msum_c, wsh


def _vt_shard(verts_shard):
    """[nvc, 3] f32 -> [4, nvc] f16 rows [vx; vy; vz; 1]."""
    nvc = verts_shard.shape[0]
    vt = np.ones((4, nvc), np.float16)
    vt[0:3] = verts_shard.T.astype(np.float16)
    return np.ascontiguousarray(vt)


def make_in_maps(inputs):
    table, msum_a, msum_b, msum_c, wsh = _host_prep(
        inputs["x"], inputs["W_shift"], inputs["b_shift"], inputs["W_diff"],
        inputs["b_diff"], inputs["W_center"], inputs["b_center"],
        inputs["W_sum"], inputs["b_sum"])
    in_maps = []
    for core in range(8):
        b, h = divmod(core, 2)
        vs = np.ascontiguousarray(
            inputs["vertices"][b, h * NVC : (h + 1) * NVC]).astype(np.float32)
        in_maps.append({
            "verts": vs,
            "vt": _vt_shard(vs),
            "table": table[b],
            "msum_a": msum_a, "msum_b": msum_b, "msum_c": msum_c,
            "wsh_a": np.ascontiguousarray(wsh[0:128]),
            "wsh_b": np.ascontiguousarray(wsh[128:256]),
            "wsh_c": np.ascontiguousarray(wsh[256:260]),
            "rep16": np.tile(np.eye(16, dtype=np.float32), 8),
        })
    return in_maps


def kernel(x, vertices, W_shift, b_shift, W_diff, b_diff, W_center, b_center,
           W_sum, b_sum):
    if "nc" not in _CACHED:
        _CACHED["nc"] = build_program()
    nc = _CACHED["nc"]

    in_maps = make_in_maps(dict(
        x=x, vertices=vertices, W_shift=W_shift, b_shift=b_shift,
        W_diff=W_diff, b_diff=b_diff, W_center=W_center, b_center=b_center,
        W_sum=W_sum, b_sum=b_sum))

    res = run_bass_kernel_spmd(nc, in_maps, core_ids=list(range(8)))
    out = np.empty((B, N, C), np.float32)
    for core in range(8):
        b, h = divmod(core, 2)
        out[b, h * NVC : (h + 1) * NVC] = _core_out(res.results[core]["out"])
    return out


def _core_out(raw):
    """[2, 128, nvc] transposed device output -> [nvc, 256]."""
    return raw.reshape(2 * 128, -1).T
